# revision 18
# baseline (speedup 1.0000x reference)
"""Trainium2 Bass kernel for nn_AttentionModule (channel-attention block).

Reference computation (per example):
    q = wq @ x + bq        # [C, P]  (1x1 conv == channelwise linear)
    k = wk @ x + bk
    v = x                  # [C, P]
    att[n] = softmax((q[n] @ k[n].T) / sqrt(dh))   # [dh, dh] per head
    out = wo @ (att @ v) + bo + x

Sharding: pure data parallel -- B=16 examples, 2 per core across 8 cores;
weights replicated. No collectives.

Gram-matrix reformulation (the key FLOP cut vs the direct q/k path):
    logits = q @ k.T = wq S wk^T + bq (wk s + P bk)^T + (wq s) bk^T
  with S = x @ x^T [C, C] and s = x @ 1_P. Computing S once (C^2 P MACs)
  replaces both q and k projections (2 C^2 P) and the P-wide attention
  contraction. S is symmetric, so only upper-triangle blocks are
  computed (1280 of 2048 N-columns per p-tile); the missing blocks are
  mirrored with cheap PE transposes. Per-example MACs ~1.96e9 vs the
  direct path's ~3.54e9. The rank-2 bias correction rides into the
  logits PSUM group as one K=2 matmul of host-precomputed rows
  (bq, qs) x (ks + P bk, bk).

Kernel design (per core; bf16 matmuls, f32 PSUM):
  * S accumulates in 4 PSUM banks over 32 xT p-tiles. xT ships in a
    pair-packed layout [16, 128, 1024] (two p-tiles per SBUF tile) so
    one DMA covers two tiles with 2KB lines; early groups are striped
    across queues for startup latency.
  * S -> SBUF as hi/lo bf16 pair (hi = bf16(S), lo = bf16(S - hi), ~16
    mantissa bits) in 128-col strips. Emission order keeps the DVE
    queue short where it gates PE: hi strips (ACT) -> hi mirrors (DVE
    copies) interleaved with V-hi matmuls -> lo strips (DVE) + lo
    mirrors -> V-lo matmuls. V extracted hi/lo the same way.
  * logits per head-pair tile [128, 128]: 8 matmuls (4 ci x hi/lo) plus
    the K=2 bias matmul; two pair banks share one PSUM bank as a single
    accumulation group (start/stop on first/last matmul only).
  * softmax: exp with constant shift (exact; margins verified) on the
    two diagonal 64x64 blocks, accum_out giving row sums Z for free;
    reciprocal + per-partition scale; no PE transpose.
    G = att_de @ woT + I (eye matmul into the same PSUM group).
  * epilogue: out = (G+I)^T @ x + bo as 8 chunks x 4 co of N=512
    matmuls per example. PSUM->SBUF bias copies alternate DVE/ACT, and
    outputs drain in three stages to shorten the exit tail.
  * DMA issue is split across engine queues by phase load: Sync issues
    all xT and x1 loads, Scalar issues weights + x0 + output drains
    (each dma_start costs ~370ns of issuing-engine time; Sync alone
    saturates during the short S0 phase otherwise).
  * schedule: S1 p-tiles run during L0/G0 and interleave with conv0
    chunks; sv1 (V1 matmuls) is sandwiched between the last conv0
    chunks so the PE queue stays fed through e1's extraction chains.

Measured on trn2 (8 cores): see test.py; rel err ~8e-3 vs f32 reference.
"""

import numpy as np
import ml_dtypes

BF = np.dtype(ml_dtypes.bfloat16)

import concourse.bass as bass
import concourse.tile as tile
from concourse import bacc, mybir
from concourse import bass_utils

F32 = mybir.dt.float32
BF16 = mybir.dt.bfloat16
EXP = mybir.ActivationFunctionType.Exp
IDENT = mybir.ActivationFunctionType.Identity

B, C, HH, WW = 16, 512, 64, 64
P = HH * WW            # 4096 spatial positions
NCORES = 8
BL = B // NCORES       # 2 examples per core
NH = 8
DH = C // NH           # 64
NPT = P // 128         # 32 p-tiles (S accumulation granularity)
NPG = NPT // 2         # 16 pair-packed xT groups
NP5 = P // 512         # 8 512-wide chunks (epilogue granularity)
NCT = C // 128         # 4 channel tiles
WCOLS = NCT * C        # 2048 cols per packed weight


def build_nc():
    nc = bacc.Bacc(
        "TRN2", target_bir_lowering=False, debug=False, enable_asserts=False
    )
    xt_d = nc.dram_tensor("xt", [BL, NPG, 128, 1024], BF16,
                          kind="ExternalInput").ap()
    x_d = nc.dram_tensor("x", [BL, C, P], BF16, kind="ExternalInput").ap()
    wpack_d = nc.dram_tensor("wpack", [128, 3 * WCOLS + 192], BF16,
                             kind="ExternalInput").ap()
    bias2_d = nc.dram_tensor("bias2", [2, BL * 2 * C], BF16,
                             kind="ExternalInput").ap()
    bpack_d = nc.dram_tensor("bpack", [128, NCT], F32,
                             kind="ExternalInput").ap()
    out_d = nc.dram_tensor("out", [BL, C, P], BF16, kind="ExternalOutput").ap()

    with (
        tile.TileContext(nc) as tc,
        tc.tile_pool(name="w", bufs=1) as wpool,
        tc.tile_pool(name="xt", bufs=14) as xtpool,
        tc.tile_pool(name="x", bufs=12) as xpool,
        tc.tile_pool(name="slv", bufs=16) as slvpool,
        tc.tile_pool(name="pair", bufs=8) as pairpool,
        tc.tile_pool(name="z", bufs=16) as zpool,
        tc.tile_pool(name="g", bufs=8) as gpool,
        tc.tile_pool(name="o2r", bufs=8) as o2rpool,
        tc.tile_pool(name="sp", bufs=4, space="PSUM") as spool,
        tc.tile_pool(name="attp", bufs=1, space="PSUM") as attpool,
        tc.tile_pool(name="p2p", bufs=3, space="PSUM") as p2pool,
    ):
        # ---- resident weights / biases -------------------------------
        wk_t = wpool.tile([128, WCOLS], BF16, tag="wk")
        wq_t = wpool.tile([128, WCOLS], BF16, tag="wq")
        wo_t = wpool.tile([128, WCOLS], BF16, tag="wo")
        konst = wpool.tile([128, 192], BF16, tag="konst")
        bias2 = wpool.tile([2, BL * 2 * C], BF16, tag="bias2")
        bo_t = wpool.tile([128, NCT], F32, tag="bo")
        zblk = konst[:, 0:64]     # all-zeros [128, 64]
        eye = konst[:, 64:192]    # identity [128, 128]
        shift = wpool.tile([128, 1], F32, tag="shift")
        nc.gpsimd.memset(shift[:], -55.0)

        # weight DMAs (issued off Scalar, deferred into the S0 loop;
        # konst first: the eye feeds the mirror transposes at ~18us)
        wdmas = [(konst[:], wpack_d[:, 3 * WCOLS: 3 * WCOLS + 192]),
                 (wk_t[:], wpack_d[:, WCOLS: 2 * WCOLS]),
                 (wq_t[:], wpack_d[:, 0: WCOLS]),
                 (bias2[:], bias2_d[:]),
                 (wo_t[:], wpack_d[:, 2 * WCOLS: 3 * WCOLS]),
                 (bo_t[:], bpack_d[:])]

        def dma_xtg(e, g, tiles, stripes=1, engines=None):
            """Issue the DMA(s) for one pair-packed xT group [128, 1024]."""
            if engines is None:
                engines = (nc.sync,)
            xtt = xtpool.tile([128, 1024], BF16, tag="xt", name=f"xt{e}_{g}")
            w = 1024 // stripes
            for st in range(stripes):
                engines[st % len(engines)].dma_start(
                    xtt[:, st * w:(st + 1) * w],
                    xt_d[e, g, :, st * w:(st + 1) * w])
            tiles[g] = xtt

        def mm_s(Sb, tiles, p):
            """Upper-triangle Gram matmuls for one p-tile."""
            base = (p % 2) * 512
            xtt = tiles[p // 2]
            for ci in range(NCT):
                nc.tensor.matmul(Sb[ci][:],
                                 xtt[:, base + 128 * ci: base + 128 * (ci + 1)],
                                 xtt[:, base + 128 * ci: base + 512],
                                 start=(p == 0), stop=(p == NPT - 1))

        def mm_s_tail(Sb, tiles):
            """Last four p-tiles in ci-major order: bank ci stops several
            matmuls before bank ci+1, so hi-strip extraction (ACT)
            overlaps the S tail instead of serializing after it."""
            for ci in range(NCT):
                for p in range(NPT - 4, NPT):
                    base = (p % 2) * 512
                    xtt = tiles[p // 2]
                    nc.tensor.matmul(Sb[ci][:],
                                     xtt[:, base + 128 * ci: base + 128 * (ci + 1)],
                                     xtt[:, base + 128 * ci: base + 512],
                                     start=False, stop=(p == NPT - 1))

        def emit_x_chunk(e, xch, idx, engine):
            """One [128, 2048] load of the [C, P]-layout x (epilogue rhs)."""
            ci, c = idx % NCT, idx // NCT
            xt = xpool.tile([128, 2048], BF16, tag="x", name=f"x{e}_{ci}_{c}")
            engine.dma_start(
                xt[:], x_d[e, ci * 128:(ci + 1) * 128, c * 2048:(c + 1) * 2048])
            xch[ci][c] = xt

        def emit_sv(e, Sb):
            """S hi/lo strip extraction + symmetry mirrors + V = S @ wkT
            + V hi/lo extraction. Ordered so the DVE queue stays short
            where it gates PE (hi mirrors before the lo strip subs)."""
            Shi = [slvpool.tile([128, C], BF16, tag="slv", name=f"Shi{e}_{ci}")
                   for ci in range(NCT)]
            Slo = [slvpool.tile([128, C], BF16, tag="slv", name=f"Slo{e}_{ci}")
                   for ci in range(NCT)]

            def strips_hi(ci):
                for s in range(NCT - ci):
                    dsl = slice((ci + s) * 128, (ci + s + 1) * 128)
                    ssl = slice(s * 128, (s + 1) * 128)
                    nc.scalar.activation(Shi[ci][:, dsl], Sb[ci][:, ssl], IDENT)

            def strips_lo(ci):
                # one coarse sub per bank: fewer DVE ops on the critical
                # S->V->L chain
                dsl = slice(ci * 128, C)
                ssl = slice(0, C - ci * 128)
                nc.vector.tensor_sub(Slo[ci][:, dsl], Sb[ci][:, ssl],
                                     Shi[ci][:, dsl])

            def mirror(pair, nm, i, j):
                # S[i-block, j-block] = S[j-block, i-block]^T for j < i;
                # hi copies on DVE, lo copies on ACT (queue balance)
                tp = p2pool.tile([128, 128], BF16, tag="p2",
                                 name=f"mt{nm}{e}_{i}{j}")
                nc.tensor.transpose(tp[:], pair[j][:, 128 * i:128 * (i + 1)],
                                    eye[:])
                if nm == "h":
                    nc.vector.tensor_copy(pair[i][:, 128 * j:128 * (j + 1)],
                                          tp[:])
                else:
                    nc.scalar.activation(pair[i][:, 128 * j:128 * (j + 1)],
                                         tp[:], IDENT)

            Vb = [spool.tile([128, 512], F32, tag="sp", name=f"V{e}_{ci}")
                  for ci in range(NCT)]

            def vmm(cj, hi):
                src = Shi if hi else Slo
                for ci in range(NCT):
                    nc.tensor.matmul(Vb[ci][:],
                                     src[cj][:, 128 * ci:128 * (ci + 1)],
                                     wk_t[:, C * cj:C * (cj + 1)],
                                     start=(hi and cj == 0),
                                     stop=((not hi) and cj == NCT - 1))

            # V banks reuse the S banks, each freed by its LAST reader
            # (the lo sub) -- so subs lead the hi mirrors on DVE and the
            # V-hi matmuls are not gated on the whole lo extraction
            for ci in range(NCT):
                strips_hi(ci)
            strips_lo(0)
            mirror(Shi, "h", 1, 0)
            vmm(0, True)
            strips_lo(1)
            mirror(Shi, "h", 2, 0)
            mirror(Shi, "h", 2, 1)
            vmm(1, True)
            strips_lo(2)
            mirror(Shi, "h", 3, 0)
            mirror(Shi, "h", 3, 1)
            mirror(Shi, "h", 3, 2)
            vmm(2, True)
            strips_lo(3)
            vmm(3, True)
            mirror(Slo, "l", 1, 0)
            vmm(0, False)
            mirror(Slo, "l", 2, 0)
            mirror(Slo, "l", 2, 1)
            vmm(1, False)
            mirror(Slo, "l", 3, 0)
            mirror(Slo, "l", 3, 1)
            mirror(Slo, "l", 3, 2)
            vmm(2, False)
            vmm(3, False)

            Vhi = [slvpool.tile([128, C], BF16, tag="slv", name=f"Vhi{e}_{ci}")
                   for ci in range(NCT)]
            Vlo = [slvpool.tile([128, C], BF16, tag="slv", name=f"Vlo{e}_{ci}")
                   for ci in range(NCT)]
            for ci in range(NCT):
                for s in range(2):
                    sl = slice(s * 256, (s + 1) * 256)
                    nc.scalar.activation(Vhi[ci][:, sl], Vb[ci][:, sl], IDENT)
                nc.vector.tensor_sub(Vlo[ci][:], Vb[ci][:], Vhi[ci][:])
            return Vhi, Vlo

        def emit_logits(e, Vhi, Vlo):
            """Per head-pair logit banks [d, e']: wq^T V + rank-2 bias.
            Two pair banks share one PSUM bank (2KB) as one group."""
            bt = attpool.tile([128, 512], F32, tag="attp", name=f"Lb{e}")
            banks = [bt[:, t * 128:(t + 1) * 128] for t in range(4)]
            for cj in range(NCT):
                for t in range(4):
                    nc.tensor.matmul(banks[t][:],
                                     wq_t[:, C * cj + 128 * t: C * cj + 128 * (t + 1)],
                                     Vhi[cj][:, 128 * t:128 * (t + 1)],
                                     start=(cj == 0 and t == 0), stop=False)
            for cj in range(NCT):
                for t in range(4):
                    nc.tensor.matmul(banks[t][:],
                                     wq_t[:, C * cj + 128 * t: C * cj + 128 * (t + 1)],
                                     Vlo[cj][:, 128 * t:128 * (t + 1)],
                                     start=False, stop=False)
            for t in range(4):
                nc.tensor.matmul(banks[t][:],
                                 bias2[:, e * 2 * C + 128 * t: e * 2 * C + 128 * (t + 1)],
                                 bias2[:, e * 2 * C + C + 128 * t: e * 2 * C + C + 128 * (t + 1)],
                                 start=False, stop=(t == 3))
            return banks

        def emit_softmax_g(e, banks):
            gs = []
            for t in range(4):
                bank = banks[t]
                pr = pairpool.tile([128, 128], BF16, tag="pair", name=f"pr{e}_{t}")
                z = zpool.tile([128, 1], F32, tag="z", name=f"z{e}_{t}")
                nc.scalar.activation(pr[0:64, 0:64], bank[0:64, 0:64], EXP,
                                     scale=0.125, bias=shift[0:64, :],
                                     accum_out=z[0:64, :])
                nc.scalar.activation(pr[64:128, 64:128], bank[64:128, 64:128],
                                     EXP, scale=0.125, bias=shift[64:128, :],
                                     accum_out=z[64:128, :])
                nc.vector.tensor_copy(pr[0:64, 64:128], zblk[0:64, :])
                nc.vector.tensor_copy(pr[64:128, 0:64], zblk[64:128, :])
                rz = zpool.tile([128, 1], F32, tag="z", name=f"rz{e}_{t}")
                nc.vector.reciprocal(rz[:], z[:])
                att_de = pairpool.tile([128, 128], BF16, tag="pair",
                                       name=f"attde{e}_{t}")
                nc.vector.tensor_scalar_mul(att_de[:], pr[:], rz[:, 0:1])
                gp = p2pool.tile([128, 512], F32, tag="p2", name=f"gp{e}_{t}")
                nc.tensor.matmul(gp[:], att_de[:], wo_t[:, C * t:C * (t + 1)],
                                 start=True, stop=False)
                nc.tensor.matmul(gp[:, 128 * t:128 * (t + 1)], eye[:], eye[:],
                                 start=False, stop=True)
                g = gpool.tile([128, C], BF16, tag="g", name=f"g{e}_{t}")
                nc.vector.tensor_copy(g[:], gp[:])
                gs.append(g)
            return gs

        def emit_o2rows(e):
            return [o2rpool.tile([128, P], BF16, tag="o2r", name=f"o2r{e}_{co}")
                    for co in range(NCT)]

        def emit_conv_chunk(e, xch, gs, o2rows, p5):
            sl = slice(p5 * 512, (p5 + 1) * 512)
            for co in range(NCT):
                o2p = p2pool.tile([128, 512], F32, tag="p2",
                                  name=f"o2p{e}_{p5}_{co}")
                for et in range(NCT):
                    nc.tensor.matmul(
                        o2p[:],
                        gs[et][:, co * 128:(co + 1) * 128],
                        xch[et][p5 // 4][:, (p5 % 4) * 512:(p5 % 4) * 512 + 512],
                        start=(et == 0), stop=(et == NCT - 1))
                # PSUM->SBUF + bias split between DVE and ACT
                if (p5 * NCT + co) % 2 == 0:
                    nc.vector.tensor_scalar_add(o2rows[co][:, sl], o2p[:],
                                                bo_t[:, co:co + 1])
                else:
                    nc.scalar.activation(o2rows[co][:, sl], o2p[:], IDENT,
                                         bias=bo_t[:, co:co + 1])
                # last two stages drain per-co right behind the copy so
                # the exit tail is one 64KB transfer per queue deep
                if p5 >= 6:
                    eng = nc.sync if co % 2 == 0 else nc.scalar
                    eng.dma_start(
                        out_d[e, co * 128:(co + 1) * 128, sl], o2rows[co][:, sl])
            # staged output drains; final stages spread across the
            # sync/vector/scalar DMA queues for a short exit tail
            if p5 == 3:
                for co in range(NCT):
                    nc.scalar.dma_start(
                        out_d[e, co * 128:(co + 1) * 128, 0:1024],
                        o2rows[co][:, 0:1024])
                    nc.scalar.dma_start(
                        out_d[e, co * 128:(co + 1) * 128, 1024:2048],
                        o2rows[co][:, 1024:2048])
            elif p5 == 5:
                for co in range(NCT):
                    nc.scalar.dma_start(
                        out_d[e, co * 128:(co + 1) * 128, 2048:3072],
                        o2rows[co][:, 2048:3072])


        # ---- schedule -------------------------------------------------
        # e0 S phase: xT0 rides BOTH hardware DMA queues (sync+scalar);
        # weights early on scalar, x0 late on both
        Sb0 = [spool.tile([128, 512 - 128 * ci], F32, tag="sp",
                          name=f"S0_{ci}") for ci in range(NCT)]
        xt0 = {}
        xch0 = [[None] * (P // 2048) for _ in range(NCT)]
        both = (nc.sync, nc.scalar)
        for g in range(4):
            dma_xtg(0, g, xt0, stripes=2, engines=both)
        for p in range(NPT):
            g = p // 2 + 4
            if p % 2 == 0 and g < NPG:
                dma_xtg(0, g, xt0, engines=(both[(g // 2) % 2],))
            mm_s(Sb0, xt0, p)
            if 2 <= p < 8 and wdmas:
                dst, src = wdmas.pop(0)
                nc.scalar.dma_start(dst, src)
            if 22 <= p < 30:
                emit_x_chunk(0, xch0, p - 22, both[p % 2])
        while wdmas:
            dst, src = wdmas.pop(0)
            nc.scalar.dma_start(dst, src)
        # xT1 early groups (needed from ~L0 time on)
        xt1 = {}
        for g in range(3):
            dma_xtg(1, g, xt1)

        Vhi0, Vlo0 = emit_sv(0, Sb0)
        Sb1 = [spool.tile([128, 512 - 128 * ci], F32, tag="sp",
                          name=f"S1_{ci}") for ci in range(NCT)]
        for p in range(0, 6):
            mm_s(Sb1, xt1, p)
        for g in range(3, 6):
            dma_xtg(1, g, xt1)
        banks0 = emit_logits(0, Vhi0, Vlo0)
        for p in range(6, 12):
            mm_s(Sb1, xt1, p)
        gs0 = emit_softmax_g(0, banks0)

        # conv0 chunks interleaved with remaining e1 S tiles + x1 loads;
        # e1's extraction/logit chains each ride behind a conv0 chunk
        o2r0 = emit_o2rows(0)
        xch1 = [[None] * (P // 2048) for _ in range(NCT)]
        for g in range(6, 12):
            dma_xtg(1, g, xt1)
        p1 = 12
        x1_next = 0
        sched = [4, 4, 4, 4, 4, 0, 0, 0]
        for i in range(NP5):
            emit_conv_chunk(0, xch0, gs0, o2r0, i)
            if i < 4:
                dma_xtg(1, 12 + i, xt1)
            for _ in range(sched[i]):
                if p1 < NPT:
                    mm_s(Sb1, xt1, p1)
                    p1 += 1
            if x1_next < 8:
                emit_x_chunk(1, xch1, x1_next, nc.sync)
                x1_next += 1
            if i == 5:
                Vhi1, Vlo1 = emit_sv(1, Sb1)
            elif i == 6:
                banks1 = emit_logits(1, Vhi1, Vlo1)

        gs1 = emit_softmax_g(1, banks1)
        o2r1 = emit_o2rows(1)
        for i in range(NP5):
            emit_conv_chunk(1, xch1, gs1, o2r1, i)

    nc.compile()
    return nc


_NC_CACHE = None


def _get_nc():
    global _NC_CACHE
    if _NC_CACHE is None:
        _NC_CACHE = build_nc()
    return _NC_CACHE


def make_in_maps(inputs):
    x = np.ascontiguousarray(np.asarray(inputs["x"], dtype=np.float32))
    wq = np.asarray(inputs["wq"], dtype=np.float32)
    wk = np.asarray(inputs["wk"], dtype=np.float32)
    wo = np.asarray(inputs["wo"], dtype=np.float32)
    bq = np.asarray(inputs["bq"], dtype=np.float32)
    bk = np.asarray(inputs["bk"], dtype=np.float32)
    bo = np.asarray(inputs["bo"], dtype=np.float32)

    x32 = x.reshape(B, C, P)
    xr = x32.astype(BF)                                   # [B, C, P] bf16
    xtr = np.ascontiguousarray(xr.transpose(0, 2, 1))     # [B, P, C] bf16
    # pair-packed xT: [B, NPG, 128, 1024], group g = p-tiles 2g, 2g+1
    xt4 = np.ascontiguousarray(
        xtr.reshape(B, NPG, 2, 128, C).transpose(0, 1, 3, 2, 4)
           .reshape(B, NPG, 128, 1024))

    # rank-2 bias-correction rows (exact f32 host math)
    s = x32.sum(axis=2)                                   # [B, C]
    qs = s @ wq.T                                         # [B, C]
    ks = s @ wk.T                                         # [B, C]

    wpack = np.zeros((128, 3 * WCOLS + 192), dtype=BF)
    for i, w in enumerate((wq, wk, wo)):
        wt = w.T.astype(BF)  # [ci, co]
        for ci in range(NCT):
            wpack[:, i * WCOLS + ci * C: i * WCOLS + (ci + 1) * C] = \
                wt[ci * 128:(ci + 1) * 128, :]
    ko = 3 * WCOLS
    wpack[:, ko + 64: ko + 192] = np.eye(128, dtype=np.float32).astype(BF)

    bpack = bo.reshape(NCT, 128).T.astype(np.float32)
    bpack = np.ascontiguousarray(bpack)

    in_maps = []
    for cix in range(NCORES):
        bias2 = np.zeros((2, BL * 2 * C), dtype=BF)
        for e in range(BL):
            ge = cix * BL + e
            bias2[0, e * 2 * C: e * 2 * C + C] = bq.astype(BF)
            bias2[1, e * 2 * C: e * 2 * C + C] = qs[ge].astype(BF)
            bias2[0, e * 2 * C + C: (e + 1) * 2 * C] = (ks[ge] + P * bk).astype(BF)
            bias2[1, e * 2 * C + C: (e + 1) * 2 * C] = bk.astype(BF)
        in_maps.append({
            "x": np.ascontiguousarray(xr[cix * BL: (cix + 1) * BL]),
            "xt": np.ascontiguousarray(xt4[cix * BL: (cix + 1) * BL]),
            "wpack": wpack, "bias2": bias2, "bpack": bpack,
        })
    return in_maps


def run_sharded(inputs, trace=False, **kw):
    nc = _get_nc()
    in_maps = make_in_maps(inputs)
    res = bass_utils.run_bass_kernel_spmd(
        nc, in_maps, core_ids=list(range(NCORES)), trace=trace, **kw
    )
    outs = [np.asarray(res.results[i]["out"]).astype(np.float32)
            for i in range(NCORES)]
    full = np.concatenate(outs, axis=0).reshape(B, C, HH, WW)
    return full.astype(np.float32), res


def kernel(**inputs):
    out, _ = run_sharded(inputs, trace=False)
    return out


# revision 19
# speedup vs baseline: 1.1344x; 1.1344x over previous
"""Trainium2 Bass kernel for nn_AttentionModule (channel-attention block).

Reference computation (per example):
    q = wq @ x + bq        # [C, P]  (1x1 conv == channelwise linear)
    k = wk @ x + bk
    v = x                  # [C, P]
    att[n] = softmax((q[n] @ k[n].T) / sqrt(dh))   # [dh, dh] per head
    out = wo @ (att @ v) + bo + x

Sharding: pure data parallel -- B=16 examples, 2 per core across 8 cores;
weights replicated. No collectives.

Gram-matrix reformulation (the key FLOP cut vs the direct q/k path):
    logits = q @ k.T = wq S wk^T + bq (wk s + P bk)^T + (wq s) bk^T
  with S = x @ x^T [C, C] and s = x @ 1_P. Computing S once (C^2 P MACs)
  replaces both q and k projections (2 C^2 P) and the P-wide attention
  contraction. S is symmetric, so only upper-triangle blocks are
  computed (1280 of 2048 N-columns per p-tile); the missing blocks are
  mirrored with cheap PE transposes. Per-example MACs ~1.96e9 vs the
  direct path's ~3.54e9. The rank-2 bias correction rides into the
  logits PSUM group as one K=2 matmul of host-precomputed rows
  (bq, qs) x (ks + P bk, bk).

Kernel design (per core; bf16 matmuls, f32 PSUM):
  * S accumulates in 4 PSUM banks over 32 xT p-tiles. xT ships in a
    pair-packed layout [16, 128, 1024] (two p-tiles per SBUF tile) so
    one DMA covers two tiles with 2KB lines; early groups are striped
    across queues for startup latency.
  * S -> SBUF as hi/lo bf16 pair (hi = bf16(S), lo = bf16(S - hi), ~16
    mantissa bits) in 128-col strips. Emission order keeps the DVE
    queue short where it gates PE: hi strips (ACT) -> hi mirrors (DVE
    copies) interleaved with V-hi matmuls -> lo strips (DVE) + lo
    mirrors -> V-lo matmuls. V extracted hi/lo the same way.
  * logits per head-pair tile [128, 128]: 8 matmuls (4 ci x hi/lo) plus
    the K=2 bias matmul; two pair banks share one PSUM bank as a single
    accumulation group (start/stop on first/last matmul only).
  * softmax: exp with constant shift (exact; margins verified) on the
    two diagonal 64x64 blocks, accum_out giving row sums Z for free;
    reciprocal + per-partition scale; no PE transpose.
    G = att_de @ woT + I (eye matmul into the same PSUM group).
  * epilogue: out = (G+I)^T @ x + bo as 8 chunks x 4 co of N=512
    matmuls per example. PSUM->SBUF bias copies alternate DVE/ACT, and
    outputs drain in three stages to shorten the exit tail.
  * DMA issue is split across engine queues by phase load: Sync issues
    all xT and x1 loads, Scalar issues weights + x0 + output drains
    (each dma_start costs ~370ns of issuing-engine time; Sync alone
    saturates during the short S0 phase otherwise).
  * schedule: S1 p-tiles run during L0/G0 and interleave with conv0
    chunks; sv1 (V1 matmuls) is sandwiched between the last conv0
    chunks so the PE queue stays fed through e1's extraction chains.

Measured on trn2 (8 cores): see test.py; rel err ~8e-3 vs f32 reference.
"""

import numpy as np
import ml_dtypes

BF = np.dtype(ml_dtypes.bfloat16)

import concourse.bass as bass
import concourse.tile as tile
from concourse import bacc, mybir
from concourse import bass_utils

F32 = mybir.dt.float32
BF16 = mybir.dt.bfloat16
EXP = mybir.ActivationFunctionType.Exp
IDENT = mybir.ActivationFunctionType.Identity

B, C, HH, WW = 16, 512, 64, 64
P = HH * WW            # 4096 spatial positions
NCORES = 8
BL = B // NCORES       # 2 examples per core
NH = 8
DH = C // NH           # 64
NPT = P // 128         # 32 p-tiles (S accumulation granularity)
NPG = NPT // 2         # 16 pair-packed xT groups
NP5 = P // 512         # 8 512-wide chunks (epilogue granularity)
NCT = C // 128         # 4 channel tiles
WCOLS = NCT * C        # 2048 cols per packed weight


def build_nc():
    nc = bacc.Bacc(
        "TRN2", target_bir_lowering=False, debug=False, enable_asserts=False
    )
    xt_d = nc.dram_tensor("xt", [BL, NPG, 128, 1024], BF16,
                          kind="ExternalInput").ap()
    x_d = nc.dram_tensor("x", [BL, C, P], BF16, kind="ExternalInput").ap()
    wpack_d = nc.dram_tensor("wpack", [128, 3 * WCOLS + 192], BF16,
                             kind="ExternalInput").ap()
    bias2_d = nc.dram_tensor("bias2", [2, BL * 2 * C], BF16,
                             kind="ExternalInput").ap()
    bpack_d = nc.dram_tensor("bpack", [128, NCT], F32,
                             kind="ExternalInput").ap()
    out_d = nc.dram_tensor("out", [BL, C, P], BF16, kind="ExternalOutput").ap()

    with (
        tile.TileContext(nc) as tc,
        tc.tile_pool(name="w", bufs=1) as wpool,
        tc.tile_pool(name="xt", bufs=14) as xtpool,
        tc.tile_pool(name="x", bufs=12) as xpool,
        tc.tile_pool(name="slv", bufs=8) as slvpool,
        tc.tile_pool(name="pair", bufs=8) as pairpool,
        tc.tile_pool(name="z", bufs=16) as zpool,
        tc.tile_pool(name="g", bufs=8) as gpool,
        tc.tile_pool(name="o2r", bufs=8) as o2rpool,
        tc.tile_pool(name="sp", bufs=4, space="PSUM") as spool,
        tc.tile_pool(name="attp", bufs=1, space="PSUM") as attpool,
        tc.tile_pool(name="p2p", bufs=3, space="PSUM") as p2pool,
    ):
        # ---- resident weights / biases -------------------------------
        wk_t = wpool.tile([128, WCOLS], BF16, tag="wk")
        wq_t = wpool.tile([128, WCOLS], BF16, tag="wq")
        wo_t = wpool.tile([128, WCOLS], BF16, tag="wo")
        konst = wpool.tile([128, 192], BF16, tag="konst")
        bias2 = wpool.tile([2, BL * 2 * C], BF16, tag="bias2")
        bo_t = wpool.tile([128, NCT], F32, tag="bo")
        zblk = konst[:, 0:64]     # all-zeros [128, 64]
        eye = konst[:, 64:192]    # identity [128, 128]
        shift = wpool.tile([128, 1], F32, tag="shift")
        nc.gpsimd.memset(shift[:], -55.0)

        # weight DMAs (issued off Scalar, deferred into the S0 loop;
        # konst first: the eye feeds the mirror transposes at ~18us)
        wdmas = [(konst[:], wpack_d[:, 3 * WCOLS: 3 * WCOLS + 192]),
                 (wk_t[:], wpack_d[:, WCOLS: 2 * WCOLS]),
                 (wq_t[:], wpack_d[:, 0: WCOLS]),
                 (bias2[:], bias2_d[:]),
                 (wo_t[:], wpack_d[:, 2 * WCOLS: 3 * WCOLS]),
                 (bo_t[:], bpack_d[:])]

        def dma_xtg(e, g, tiles, stripes=1, engines=None):
            """Issue the DMA(s) for one pair-packed xT group [128, 1024]."""
            if engines is None:
                engines = (nc.sync,)
            xtt = xtpool.tile([128, 1024], BF16, tag="xt", name=f"xt{e}_{g}")
            w = 1024 // stripes
            for st in range(stripes):
                engines[st % len(engines)].dma_start(
                    xtt[:, st * w:(st + 1) * w],
                    xt_d[e, g, :, st * w:(st + 1) * w])
            tiles[g] = xtt

        def mm_s(Sb, tiles, p):
            """Upper-triangle Gram matmuls for one p-tile."""
            base = (p % 2) * 512
            xtt = tiles[p // 2]
            for ci in range(NCT):
                nc.tensor.matmul(Sb[ci][:],
                                 xtt[:, base + 128 * ci: base + 128 * (ci + 1)],
                                 xtt[:, base + 128 * ci: base + 512],
                                 start=(p == 0), stop=(p == NPT - 1))

        def mm_s_tail(Sb, tiles):
            """Last four p-tiles in ci-major order: bank ci stops several
            matmuls before bank ci+1, so hi-strip extraction (ACT)
            overlaps the S tail instead of serializing after it."""
            for ci in range(NCT):
                for p in range(NPT - 4, NPT):
                    base = (p % 2) * 512
                    xtt = tiles[p // 2]
                    nc.tensor.matmul(Sb[ci][:],
                                     xtt[:, base + 128 * ci: base + 128 * (ci + 1)],
                                     xtt[:, base + 128 * ci: base + 512],
                                     start=False, stop=(p == NPT - 1))

        def emit_x_chunk(e, xch, idx, engine):
            """One [128, 2048] load of the [C, P]-layout x (epilogue rhs)."""
            ci, c = idx % NCT, idx // NCT
            xt = xpool.tile([128, 2048], BF16, tag="x", name=f"x{e}_{ci}_{c}")
            engine.dma_start(
                xt[:], x_d[e, ci * 128:(ci + 1) * 128, c * 2048:(c + 1) * 2048])
            xch[ci][c] = xt

        def emit_sv(e, Sb):
            """S extraction (single bf16; rel err ~1.1e-2 vs tolerance
            2e-2 -- the hi/lo split costs 10K PE cycles/example for
            ~2e-3) + symmetry mirrors + V = S @ wkT + V extraction."""
            Shi = [slvpool.tile([128, C], BF16, tag="slv", name=f"Shi{e}_{ci}")
                   for ci in range(NCT)]
            for ci in range(NCT):
                for s in range(NCT - ci):
                    dsl = slice((ci + s) * 128, (ci + s + 1) * 128)
                    ssl = slice(s * 128, (s + 1) * 128)
                    nc.scalar.activation(Shi[ci][:, dsl], Sb[ci][:, ssl], IDENT)

            def mirror(i, j):
                # S[i-block, j-block] = S[j-block, i-block]^T for j < i
                tp = p2pool.tile([128, 128], BF16, tag="p2",
                                 name=f"mt{e}_{i}{j}")
                nc.tensor.transpose(tp[:], Shi[j][:, 128 * i:128 * (i + 1)],
                                    eye[:])
                nc.vector.tensor_copy(Shi[i][:, 128 * j:128 * (j + 1)], tp[:])

            Vb = [spool.tile([128, 512], F32, tag="sp", name=f"V{e}_{ci}")
                  for ci in range(NCT)]

            def vmm(cj):
                for ci in range(NCT):
                    nc.tensor.matmul(Vb[ci][:],
                                     Shi[cj][:, 128 * ci:128 * (ci + 1)],
                                     wk_t[:, C * cj:C * (cj + 1)],
                                     start=(cj == 0), stop=(cj == NCT - 1))

            mirror(1, 0)
            vmm(0)
            mirror(2, 0)
            mirror(2, 1)
            vmm(1)
            mirror(3, 0)
            mirror(3, 1)
            mirror(3, 2)
            vmm(2)
            vmm(3)

            Vhi = [slvpool.tile([128, C], BF16, tag="slv", name=f"Vhi{e}_{ci}")
                   for ci in range(NCT)]
            for ci in range(NCT):
                for s in range(2):
                    sl = slice(s * 256, (s + 1) * 256)
                    nc.scalar.activation(Vhi[ci][:, sl], Vb[ci][:, sl], IDENT)
            return Vhi

        def emit_logits(e, Vhi):
            """Per head-pair logit banks [d, e']: wq^T V + rank-2 bias.
            All four pair banks share one PSUM bank (2KB) as one group."""
            bt = attpool.tile([128, 512], F32, tag="attp", name=f"Lb{e}")
            banks = [bt[:, t * 128:(t + 1) * 128] for t in range(4)]
            for cj in range(NCT):
                for t in range(4):
                    nc.tensor.matmul(banks[t][:],
                                     wq_t[:, C * cj + 128 * t: C * cj + 128 * (t + 1)],
                                     Vhi[cj][:, 128 * t:128 * (t + 1)],
                                     start=(cj == 0 and t == 0), stop=False)
            for t in range(4):
                nc.tensor.matmul(banks[t][:],
                                 bias2[:, e * 2 * C + 128 * t: e * 2 * C + 128 * (t + 1)],
                                 bias2[:, e * 2 * C + C + 128 * t: e * 2 * C + C + 128 * (t + 1)],
                                 start=False, stop=(t == 3))
            return banks

        def emit_softmax_g(e, banks):
            gs = []
            for t in range(4):
                bank = banks[t]
                pr = pairpool.tile([128, 128], BF16, tag="pair", name=f"pr{e}_{t}")
                z = zpool.tile([128, 1], F32, tag="z", name=f"z{e}_{t}")
                nc.scalar.activation(pr[0:64, 0:64], bank[0:64, 0:64], EXP,
                                     scale=0.125, bias=shift[0:64, :],
                                     accum_out=z[0:64, :])
                nc.scalar.activation(pr[64:128, 64:128], bank[64:128, 64:128],
                                     EXP, scale=0.125, bias=shift[64:128, :],
                                     accum_out=z[64:128, :])
                nc.vector.tensor_copy(pr[0:64, 64:128], zblk[0:64, :])
                nc.vector.tensor_copy(pr[64:128, 0:64], zblk[64:128, :])
                rz = zpool.tile([128, 1], F32, tag="z", name=f"rz{e}_{t}")
                nc.vector.reciprocal(rz[:], z[:])
                att_de = pairpool.tile([128, 128], BF16, tag="pair",
                                       name=f"attde{e}_{t}")
                nc.vector.tensor_scalar_mul(att_de[:], pr[:], rz[:, 0:1])
                gp = p2pool.tile([128, 512], F32, tag="p2", name=f"gp{e}_{t}")
                nc.tensor.matmul(gp[:], att_de[:], wo_t[:, C * t:C * (t + 1)],
                                 start=True, stop=False)
                nc.tensor.matmul(gp[:, 128 * t:128 * (t + 1)], eye[:], eye[:],
                                 start=False, stop=True)
                g = gpool.tile([128, C], BF16, tag="g", name=f"g{e}_{t}")
                nc.vector.tensor_copy(g[:], gp[:])
                gs.append(g)
            return gs

        def emit_o2rows(e):
            return [o2rpool.tile([128, P], BF16, tag="o2r", name=f"o2r{e}_{co}")
                    for co in range(NCT)]

        def emit_conv_chunk(e, xch, gs, o2rows, p5):
            sl = slice(p5 * 512, (p5 + 1) * 512)
            for co in range(NCT):
                o2p = p2pool.tile([128, 512], F32, tag="p2",
                                  name=f"o2p{e}_{p5}_{co}")
                for et in range(NCT):
                    nc.tensor.matmul(
                        o2p[:],
                        gs[et][:, co * 128:(co + 1) * 128],
                        xch[et][p5 // 4][:, (p5 % 4) * 512:(p5 % 4) * 512 + 512],
                        start=(et == 0), stop=(et == NCT - 1))
                # PSUM->SBUF + bias split between DVE and ACT
                if (p5 * NCT + co) % 2 == 0:
                    nc.vector.tensor_scalar_add(o2rows[co][:, sl], o2p[:],
                                                bo_t[:, co:co + 1])
                else:
                    nc.scalar.activation(o2rows[co][:, sl], o2p[:], IDENT,
                                         bias=bo_t[:, co:co + 1])
                # last two stages drain per-co right behind the copy so
                # the exit tail is one 64KB transfer per queue deep
                if p5 >= 6:
                    eng = nc.sync if co % 2 == 0 else nc.scalar
                    eng.dma_start(
                        out_d[e, co * 128:(co + 1) * 128, sl], o2rows[co][:, sl])
            # staged output drains; final stages spread across the
            # sync/vector/scalar DMA queues for a short exit tail
            if p5 == 3:
                for co in range(NCT):
                    nc.scalar.dma_start(
                        out_d[e, co * 128:(co + 1) * 128, 0:1024],
                        o2rows[co][:, 0:1024])
                    nc.scalar.dma_start(
                        out_d[e, co * 128:(co + 1) * 128, 1024:2048],
                        o2rows[co][:, 1024:2048])
            elif p5 == 5:
                for co in range(NCT):
                    nc.scalar.dma_start(
                        out_d[e, co * 128:(co + 1) * 128, 2048:3072],
                        o2rows[co][:, 2048:3072])


        # ---- schedule -------------------------------------------------
        # e0 S phase: xT0 rides BOTH hardware DMA queues (sync+scalar);
        # weights early on scalar, x0 late on both
        Sb0 = [spool.tile([128, 512 - 128 * ci], F32, tag="sp",
                          name=f"S0_{ci}") for ci in range(NCT)]
        xt0 = {}
        xch0 = [[None] * (P // 2048) for _ in range(NCT)]
        both = (nc.sync, nc.scalar)
        for g in range(4):
            dma_xtg(0, g, xt0, stripes=2, engines=both)
        for p in range(NPT):
            g = p // 2 + 4
            if p % 2 == 0 and g < NPG:
                dma_xtg(0, g, xt0, engines=(both[(g // 2) % 2],))
            mm_s(Sb0, xt0, p)
            if 2 <= p < 8 and wdmas:
                dst, src = wdmas.pop(0)
                nc.scalar.dma_start(dst, src)
            if 22 <= p < 30:
                emit_x_chunk(0, xch0, p - 22, both[p % 2])
        while wdmas:
            dst, src = wdmas.pop(0)
            nc.scalar.dma_start(dst, src)
        # xT1 early groups (needed from ~L0 time on)
        xt1 = {}
        for g in range(3):
            dma_xtg(1, g, xt1)

        Vhi0 = emit_sv(0, Sb0)
        Sb1 = [spool.tile([128, 512 - 128 * ci], F32, tag="sp",
                          name=f"S1_{ci}") for ci in range(NCT)]
        for p in range(0, 6):
            mm_s(Sb1, xt1, p)
        for g in range(3, 6):
            dma_xtg(1, g, xt1)
        banks0 = emit_logits(0, Vhi0)
        for p in range(6, 12):
            mm_s(Sb1, xt1, p)
        gs0 = emit_softmax_g(0, banks0)

        # conv0 chunks interleaved with remaining e1 S tiles + x1 loads;
        # e1's extraction/logit chains each ride behind a conv0 chunk
        o2r0 = emit_o2rows(0)
        xch1 = [[None] * (P // 2048) for _ in range(NCT)]
        for g in range(6, 12):
            dma_xtg(1, g, xt1)
        p1 = 12
        x1_next = 0
        sched = [4, 4, 4, 4, 4, 0, 0, 0]
        for i in range(NP5):
            emit_conv_chunk(0, xch0, gs0, o2r0, i)
            if i < 4:
                dma_xtg(1, 12 + i, xt1)
            for _ in range(sched[i]):
                if p1 < NPT:
                    mm_s(Sb1, xt1, p1)
                    p1 += 1
            if x1_next < 8:
                emit_x_chunk(1, xch1, x1_next, nc.sync)
                x1_next += 1
            if i == 5:
                Vhi1 = emit_sv(1, Sb1)
            elif i == 6:
                banks1 = emit_logits(1, Vhi1)

        gs1 = emit_softmax_g(1, banks1)
        o2r1 = emit_o2rows(1)
        for i in range(NP5):
            emit_conv_chunk(1, xch1, gs1, o2r1, i)

    nc.compile()
    return nc


_NC_CACHE = None


def _get_nc():
    global _NC_CACHE
    if _NC_CACHE is None:
        _NC_CACHE = build_nc()
    return _NC_CACHE


def make_in_maps(inputs):
    x = np.ascontiguousarray(np.asarray(inputs["x"], dtype=np.float32))
    wq = np.asarray(inputs["wq"], dtype=np.float32)
    wk = np.asarray(inputs["wk"], dtype=np.float32)
    wo = np.asarray(inputs["wo"], dtype=np.float32)
    bq = np.asarray(inputs["bq"], dtype=np.float32)
    bk = np.asarray(inputs["bk"], dtype=np.float32)
    bo = np.asarray(inputs["bo"], dtype=np.float32)

    x32 = x.reshape(B, C, P)
    xr = x32.astype(BF)                                   # [B, C, P] bf16
    xtr = np.ascontiguousarray(xr.transpose(0, 2, 1))     # [B, P, C] bf16
    # pair-packed xT: [B, NPG, 128, 1024], group g = p-tiles 2g, 2g+1
    xt4 = np.ascontiguousarray(
        xtr.reshape(B, NPG, 2, 128, C).transpose(0, 1, 3, 2, 4)
           .reshape(B, NPG, 128, 1024))

    # rank-2 bias-correction rows (exact f32 host math)
    s = x32.sum(axis=2)                                   # [B, C]
    qs = s @ wq.T                                         # [B, C]
    ks = s @ wk.T                                         # [B, C]

    wpack = np.zeros((128, 3 * WCOLS + 192), dtype=BF)
    for i, w in enumerate((wq, wk, wo)):
        wt = w.T.astype(BF)  # [ci, co]
        for ci in range(NCT):
            wpack[:, i * WCOLS + ci * C: i * WCOLS + (ci + 1) * C] = \
                wt[ci * 128:(ci + 1) * 128, :]
    ko = 3 * WCOLS
    wpack[:, ko + 64: ko + 192] = np.eye(128, dtype=np.float32).astype(BF)

    bpack = bo.reshape(NCT, 128).T.astype(np.float32)
    bpack = np.ascontiguousarray(bpack)

    in_maps = []
    for cix in range(NCORES):
        bias2 = np.zeros((2, BL * 2 * C), dtype=BF)
        for e in range(BL):
            ge = cix * BL + e
            bias2[0, e * 2 * C: e * 2 * C + C] = bq.astype(BF)
            bias2[1, e * 2 * C: e * 2 * C + C] = qs[ge].astype(BF)
            bias2[0, e * 2 * C + C: (e + 1) * 2 * C] = (ks[ge] + P * bk).astype(BF)
            bias2[1, e * 2 * C + C: (e + 1) * 2 * C] = bk.astype(BF)
        in_maps.append({
            "x": np.ascontiguousarray(xr[cix * BL: (cix + 1) * BL]),
            "xt": np.ascontiguousarray(xt4[cix * BL: (cix + 1) * BL]),
            "wpack": wpack, "bias2": bias2, "bpack": bpack,
        })
    return in_maps


def run_sharded(inputs, trace=False, **kw):
    nc = _get_nc()
    in_maps = make_in_maps(inputs)
    res = bass_utils.run_bass_kernel_spmd(
        nc, in_maps, core_ids=list(range(NCORES)), trace=trace, **kw
    )
    outs = [np.asarray(res.results[i]["out"]).astype(np.float32)
            for i in range(NCORES)]
    full = np.concatenate(outs, axis=0).reshape(B, C, HH, WW)
    return full.astype(np.float32), res


def kernel(**inputs):
    out, _ = run_sharded(inputs, trace=False)
    return out


# revision 20
# speedup vs baseline: 1.1562x; 1.0192x over previous
"""Trainium2 Bass kernel for nn_AttentionModule (channel-attention block).

Reference computation (per example):
    q = wq @ x + bq        # [C, P]  (1x1 conv == channelwise linear)
    k = wk @ x + bk
    v = x                  # [C, P]
    att[n] = softmax((q[n] @ k[n].T) / sqrt(dh))   # [dh, dh] per head
    out = wo @ (att @ v) + bo + x

Sharding: pure data parallel -- B=16 examples, 2 per core across 8 cores;
weights replicated. No collectives.

Gram-matrix reformulation (the key FLOP cut vs the direct q/k path):
    logits = q @ k.T = wq S wk^T + bq (wk s + P bk)^T + (wq s) bk^T
  with S = x @ x^T [C, C] and s = x @ 1_P. Computing S once (C^2 P MACs)
  replaces both q and k projections (2 C^2 P) and the P-wide attention
  contraction. S is symmetric, so only upper-triangle blocks are
  computed (1280 of 2048 N-columns per p-tile); the missing blocks are
  mirrored with cheap PE transposes. Per-example MACs ~1.96e9 vs the
  direct path's ~3.54e9. The rank-2 bias correction rides into the
  logits PSUM group as one K=2 matmul of host-precomputed rows
  (bq, qs) x (ks + P bk, bk).

Kernel design (per core; bf16 matmuls, f32 PSUM):
  * S accumulates in 4 PSUM banks over 32 xT p-tiles. xT ships in a
    pair-packed layout [16, 128, 1024] (two p-tiles per SBUF tile) so
    one DMA covers two tiles with 2KB lines; early groups are striped
    across queues for startup latency.
  * S -> SBUF as hi/lo bf16 pair (hi = bf16(S), lo = bf16(S - hi), ~16
    mantissa bits) in 128-col strips. Emission order keeps the DVE
    queue short where it gates PE: hi strips (ACT) -> hi mirrors (DVE
    copies) interleaved with V-hi matmuls -> lo strips (DVE) + lo
    mirrors -> V-lo matmuls. V extracted hi/lo the same way.
  * logits per head-pair tile [128, 128]: 8 matmuls (4 ci x hi/lo) plus
    the K=2 bias matmul; two pair banks share one PSUM bank as a single
    accumulation group (start/stop on first/last matmul only).
  * softmax: exp with constant shift (exact; margins verified) on the
    two diagonal 64x64 blocks, accum_out giving row sums Z for free;
    reciprocal + per-partition scale; no PE transpose.
    G = att_de @ woT + I (eye matmul into the same PSUM group).
  * epilogue: out = (G+I)^T @ x + bo as 8 chunks x 4 co of N=512
    matmuls per example. PSUM->SBUF bias copies alternate DVE/ACT, and
    outputs drain in three stages to shorten the exit tail.
  * DMA issue is split across engine queues by phase load: Sync issues
    all xT and x1 loads, Scalar issues weights + x0 + output drains
    (each dma_start costs ~370ns of issuing-engine time; Sync alone
    saturates during the short S0 phase otherwise).
  * schedule: S1 p-tiles run during L0/G0 and interleave with conv0
    chunks; sv1 (V1 matmuls) is sandwiched between the last conv0
    chunks so the PE queue stays fed through e1's extraction chains.

Measured on trn2 (8 cores): see test.py; rel err ~8e-3 vs f32 reference.
"""

import numpy as np
import ml_dtypes

BF = np.dtype(ml_dtypes.bfloat16)

import concourse.bass as bass
import concourse.tile as tile
from concourse import bacc, mybir
from concourse import bass_utils

F32 = mybir.dt.float32
BF16 = mybir.dt.bfloat16
EXP = mybir.ActivationFunctionType.Exp
IDENT = mybir.ActivationFunctionType.Identity

B, C, HH, WW = 16, 512, 64, 64
P = HH * WW            # 4096 spatial positions
NCORES = 8
BL = B // NCORES       # 2 examples per core
NH = 8
DH = C // NH           # 64
NPT = P // 128         # 32 p-tiles (S accumulation granularity)
NPG = NPT // 2         # 16 pair-packed xT groups
NP5 = P // 512         # 8 512-wide chunks (epilogue granularity)
NCT = C // 128         # 4 channel tiles
WCOLS = NCT * C        # 2048 cols per packed weight


def build_nc():
    nc = bacc.Bacc(
        "TRN2", target_bir_lowering=False, debug=False, enable_asserts=False
    )
    xt_d = nc.dram_tensor("xt", [BL, NPG, 128, 1024], BF16,
                          kind="ExternalInput").ap()
    x_d = nc.dram_tensor("x", [BL, C, P], BF16, kind="ExternalInput").ap()
    wpack_d = nc.dram_tensor("wpack", [128, 3 * WCOLS + 192], BF16,
                             kind="ExternalInput").ap()
    bias2_d = nc.dram_tensor("bias2", [2, BL * 2 * C], BF16,
                             kind="ExternalInput").ap()
    bpack_d = nc.dram_tensor("bpack", [128, NCT], F32,
                             kind="ExternalInput").ap()
    out_d = nc.dram_tensor("out", [BL, C, P], BF16, kind="ExternalOutput").ap()

    with (
        tile.TileContext(nc) as tc,
        tc.tile_pool(name="w", bufs=1) as wpool,
        tc.tile_pool(name="xt", bufs=14) as xtpool,
        tc.tile_pool(name="x", bufs=12) as xpool,
        tc.tile_pool(name="slv", bufs=8) as slvpool,
        tc.tile_pool(name="pair", bufs=8) as pairpool,
        tc.tile_pool(name="z", bufs=16) as zpool,
        tc.tile_pool(name="g", bufs=8) as gpool,
        tc.tile_pool(name="o2r", bufs=8) as o2rpool,
        tc.tile_pool(name="sp", bufs=4, space="PSUM") as spool,
        tc.tile_pool(name="attp", bufs=1, space="PSUM") as attpool,
        tc.tile_pool(name="p2p", bufs=3, space="PSUM") as p2pool,
    ):
        # ---- resident weights / biases -------------------------------
        wk_t = wpool.tile([128, WCOLS], BF16, tag="wk")
        wq_t = wpool.tile([128, WCOLS], BF16, tag="wq")
        wo_t = wpool.tile([128, WCOLS], BF16, tag="wo")
        konst = wpool.tile([128, 192], BF16, tag="konst")
        bias2 = wpool.tile([2, BL * 2 * C], BF16, tag="bias2")
        bo_t = wpool.tile([128, NCT], F32, tag="bo")
        zblk = konst[:, 0:64]     # all-zeros [128, 64]
        eye = konst[:, 64:192]    # identity [128, 128]
        shift = wpool.tile([128, 1], F32, tag="shift")
        nc.gpsimd.memset(shift[:], -55.0)

        # weight DMAs (issued off Scalar, deferred into the S0 loop;
        # konst first: the eye feeds the mirror transposes at ~18us)
        wdmas = [(konst[:], wpack_d[:, 3 * WCOLS: 3 * WCOLS + 192]),
                 (wk_t[:], wpack_d[:, WCOLS: 2 * WCOLS]),
                 (wq_t[:], wpack_d[:, 0: WCOLS]),
                 (bias2[:], bias2_d[:]),
                 (wo_t[:], wpack_d[:, 2 * WCOLS: 3 * WCOLS]),
                 (bo_t[:], bpack_d[:])]

        def dma_xtg(e, g, tiles, stripes=1, engines=None):
            """Issue the DMA(s) for one pair-packed xT group [128, 1024]."""
            if engines is None:
                engines = (nc.sync,)
            xtt = xtpool.tile([128, 1024], BF16, tag="xt", name=f"xt{e}_{g}")
            w = 1024 // stripes
            for st in range(stripes):
                engines[st % len(engines)].dma_start(
                    xtt[:, st * w:(st + 1) * w],
                    xt_d[e, g, :, st * w:(st + 1) * w])
            tiles[g] = xtt

        def mm_s(Sb, tiles, p):
            """Upper-triangle Gram matmuls for one p-tile."""
            base = (p % 2) * 512
            xtt = tiles[p // 2]
            for ci in range(NCT):
                nc.tensor.matmul(Sb[ci][:],
                                 xtt[:, base + 128 * ci: base + 128 * (ci + 1)],
                                 xtt[:, base + 128 * ci: base + 512],
                                 start=(p == 0), stop=(p == NPT - 1))

        def mm_s_tail(Sb, tiles):
            """Last four p-tiles in ci-major order: bank ci stops several
            matmuls before bank ci+1, so hi-strip extraction (ACT)
            overlaps the S tail instead of serializing after it."""
            for ci in range(NCT):
                for p in range(NPT - 4, NPT):
                    base = (p % 2) * 512
                    xtt = tiles[p // 2]
                    nc.tensor.matmul(Sb[ci][:],
                                     xtt[:, base + 128 * ci: base + 128 * (ci + 1)],
                                     xtt[:, base + 128 * ci: base + 512],
                                     start=False, stop=(p == NPT - 1))

        def emit_x_chunk(e, xch, idx, engine):
            """One [128, 2048] load of the [C, P]-layout x (epilogue rhs)."""
            ci, c = idx % NCT, idx // NCT
            xt = xpool.tile([128, 2048], BF16, tag="x", name=f"x{e}_{ci}_{c}")
            engine.dma_start(
                xt[:], x_d[e, ci * 128:(ci + 1) * 128, c * 2048:(c + 1) * 2048])
            xch[ci][c] = xt

        def emit_sv(e, Sb):
            """S extraction (single bf16; rel err ~1.1e-2 vs tolerance
            2e-2 -- the hi/lo split costs 10K PE cycles/example for
            ~2e-3) + symmetry mirrors + V = S @ wkT + V extraction."""
            Shi = [slvpool.tile([128, C], BF16, tag="slv", name=f"Shi{e}_{ci}")
                   for ci in range(NCT)]
            # strips split across ACT and DVE so extraction wall-time
            # halves (both engines can read PSUM; GpSimd cannot)
            for ci in range(NCT):
                for s in range(NCT - ci):
                    dsl = slice((ci + s) * 128, (ci + s + 1) * 128)
                    ssl = slice(s * 128, (s + 1) * 128)
                    if (ci + s) % 2 == 0:
                        nc.scalar.activation(Shi[ci][:, dsl], Sb[ci][:, ssl],
                                             IDENT)
                    else:
                        nc.vector.tensor_copy(Shi[ci][:, dsl], Sb[ci][:, ssl])

            def mirror(i, j):
                # S[i-block, j-block] = S[j-block, i-block]^T for j < i
                tp = p2pool.tile([128, 128], BF16, tag="p2",
                                 name=f"mt{e}_{i}{j}")
                nc.tensor.transpose(tp[:], Shi[j][:, 128 * i:128 * (i + 1)],
                                    eye[:])
                nc.vector.tensor_copy(Shi[i][:, 128 * j:128 * (j + 1)], tp[:])

            Vb = [spool.tile([128, 512], F32, tag="sp", name=f"V{e}_{ci}")
                  for ci in range(NCT)]

            def vmm(cj):
                for ci in range(NCT):
                    nc.tensor.matmul(Vb[ci][:],
                                     Shi[cj][:, 128 * ci:128 * (ci + 1)],
                                     wk_t[:, C * cj:C * (cj + 1)],
                                     start=(cj == 0), stop=(cj == NCT - 1))

            mirror(1, 0)
            vmm(0)
            mirror(2, 0)
            mirror(2, 1)
            vmm(1)
            mirror(3, 0)
            mirror(3, 1)
            mirror(3, 2)
            vmm(2)
            vmm(3)

            Vhi = [slvpool.tile([128, C], BF16, tag="slv", name=f"Vhi{e}_{ci}")
                   for ci in range(NCT)]
            for ci in range(NCT):
                for s in range(2):
                    sl = slice(s * 256, (s + 1) * 256)
                    if (ci + s) % 2 == 0:
                        nc.scalar.activation(Vhi[ci][:, sl], Vb[ci][:, sl],
                                             IDENT)
                    else:
                        nc.vector.tensor_copy(Vhi[ci][:, sl], Vb[ci][:, sl])
            return Vhi

        def emit_logits(e, Vhi):
            """Per head-pair logit banks [d, e']: wq^T V + rank-2 bias.
            All four pair banks share one PSUM bank (2KB) as one group."""
            bt = attpool.tile([128, 512], F32, tag="attp", name=f"Lb{e}")
            banks = [bt[:, t * 128:(t + 1) * 128] for t in range(4)]
            for cj in range(NCT):
                for t in range(4):
                    nc.tensor.matmul(banks[t][:],
                                     wq_t[:, C * cj + 128 * t: C * cj + 128 * (t + 1)],
                                     Vhi[cj][:, 128 * t:128 * (t + 1)],
                                     start=(cj == 0 and t == 0), stop=False)
            for t in range(4):
                nc.tensor.matmul(banks[t][:],
                                 bias2[:, e * 2 * C + 128 * t: e * 2 * C + 128 * (t + 1)],
                                 bias2[:, e * 2 * C + C + 128 * t: e * 2 * C + C + 128 * (t + 1)],
                                 start=False, stop=(t == 3))
            return banks

        def emit_softmax_g(e, banks):
            gs = []
            for t in range(4):
                bank = banks[t]
                pr = pairpool.tile([128, 128], BF16, tag="pair", name=f"pr{e}_{t}")
                z = zpool.tile([128, 1], F32, tag="z", name=f"z{e}_{t}")
                nc.scalar.activation(pr[0:64, 0:64], bank[0:64, 0:64], EXP,
                                     scale=0.125, bias=shift[0:64, :],
                                     accum_out=z[0:64, :])
                nc.scalar.activation(pr[64:128, 64:128], bank[64:128, 64:128],
                                     EXP, scale=0.125, bias=shift[64:128, :],
                                     accum_out=z[64:128, :])
                nc.vector.tensor_copy(pr[0:64, 64:128], zblk[0:64, :])
                nc.vector.tensor_copy(pr[64:128, 0:64], zblk[64:128, :])
                rz = zpool.tile([128, 1], F32, tag="z", name=f"rz{e}_{t}")
                nc.vector.reciprocal(rz[:], z[:])
                att_de = pairpool.tile([128, 128], BF16, tag="pair",
                                       name=f"attde{e}_{t}")
                nc.vector.tensor_scalar_mul(att_de[:], pr[:], rz[:, 0:1])
                gp = p2pool.tile([128, 512], F32, tag="p2", name=f"gp{e}_{t}")
                nc.tensor.matmul(gp[:], att_de[:], wo_t[:, C * t:C * (t + 1)],
                                 start=True, stop=False)
                nc.tensor.matmul(gp[:, 128 * t:128 * (t + 1)], eye[:], eye[:],
                                 start=False, stop=True)
                g = gpool.tile([128, C], BF16, tag="g", name=f"g{e}_{t}")
                nc.vector.tensor_copy(g[:], gp[:])
                gs.append(g)
            return gs

        def emit_o2rows(e):
            return [o2rpool.tile([128, P], BF16, tag="o2r", name=f"o2r{e}_{co}")
                    for co in range(NCT)]

        def emit_conv_chunk(e, xch, gs, o2rows, p5):
            sl = slice(p5 * 512, (p5 + 1) * 512)
            for co in range(NCT):
                o2p = p2pool.tile([128, 512], F32, tag="p2",
                                  name=f"o2p{e}_{p5}_{co}")
                for et in range(NCT):
                    nc.tensor.matmul(
                        o2p[:],
                        gs[et][:, co * 128:(co + 1) * 128],
                        xch[et][p5 // 4][:, (p5 % 4) * 512:(p5 % 4) * 512 + 512],
                        start=(et == 0), stop=(et == NCT - 1))
                # PSUM->SBUF + bias split between DVE and ACT
                if (p5 * NCT + co) % 2 == 0:
                    nc.vector.tensor_scalar_add(o2rows[co][:, sl], o2p[:],
                                                bo_t[:, co:co + 1])
                else:
                    nc.scalar.activation(o2rows[co][:, sl], o2p[:], IDENT,
                                         bias=bo_t[:, co:co + 1])
                # last two stages drain per-co right behind the copy so
                # the exit tail is one 64KB transfer per queue deep
                if p5 >= 6:
                    eng = nc.sync if co % 2 == 0 else nc.scalar
                    eng.dma_start(
                        out_d[e, co * 128:(co + 1) * 128, sl], o2rows[co][:, sl])
            # staged output drains; final stages spread across the
            # sync/vector/scalar DMA queues for a short exit tail
            if p5 == 3:
                for co in range(NCT):
                    nc.scalar.dma_start(
                        out_d[e, co * 128:(co + 1) * 128, 0:1024],
                        o2rows[co][:, 0:1024])
                    nc.scalar.dma_start(
                        out_d[e, co * 128:(co + 1) * 128, 1024:2048],
                        o2rows[co][:, 1024:2048])
            elif p5 == 5:
                for co in range(NCT):
                    nc.scalar.dma_start(
                        out_d[e, co * 128:(co + 1) * 128, 2048:3072],
                        o2rows[co][:, 2048:3072])


        # ---- schedule -------------------------------------------------
        # e0 S phase: xT0 rides BOTH hardware DMA queues (sync+scalar);
        # weights early on scalar, x0 late on both
        Sb0 = [spool.tile([128, 512 - 128 * ci], F32, tag="sp",
                          name=f"S0_{ci}") for ci in range(NCT)]
        xt0 = {}
        xch0 = [[None] * (P // 2048) for _ in range(NCT)]
        both = (nc.sync, nc.scalar)
        for g in range(2):
            dma_xtg(0, g, xt0, stripes=4, engines=both)
        for g in range(2, 4):
            dma_xtg(0, g, xt0, stripes=2, engines=both)
        for p in range(NPT):
            g = p // 2 + 4
            if p % 2 == 0 and g < NPG:
                dma_xtg(0, g, xt0, engines=(both[(g // 2) % 2],))
            mm_s(Sb0, xt0, p)
            if 2 <= p < 8 and wdmas:
                dst, src = wdmas.pop(0)
                nc.scalar.dma_start(dst, src)
            if 22 <= p < 30:
                emit_x_chunk(0, xch0, p - 22, both[p % 2])
        while wdmas:
            dst, src = wdmas.pop(0)
            nc.scalar.dma_start(dst, src)
        # xT1 early groups (needed from ~L0 time on)
        xt1 = {}
        for g in range(3):
            dma_xtg(1, g, xt1)

        Vhi0 = emit_sv(0, Sb0)
        Sb1 = [spool.tile([128, 512 - 128 * ci], F32, tag="sp",
                          name=f"S1_{ci}") for ci in range(NCT)]
        for p in range(0, 6):
            mm_s(Sb1, xt1, p)
        for g in range(3, 6):
            dma_xtg(1, g, xt1)
        banks0 = emit_logits(0, Vhi0)
        for p in range(6, 12):
            mm_s(Sb1, xt1, p)
        gs0 = emit_softmax_g(0, banks0)

        # conv0 chunks interleaved with remaining e1 S tiles + x1 loads;
        # e1's extraction/logit chains each ride behind a conv0 chunk
        o2r0 = emit_o2rows(0)
        xch1 = [[None] * (P // 2048) for _ in range(NCT)]
        for g in range(6, 12):
            dma_xtg(1, g, xt1)
        p1 = 12
        x1_next = 0
        sched = [4, 4, 4, 4, 4, 0, 0, 0]
        for i in range(NP5):
            emit_conv_chunk(0, xch0, gs0, o2r0, i)
            if i < 4:
                dma_xtg(1, 12 + i, xt1)
            for _ in range(sched[i]):
                if p1 < NPT:
                    mm_s(Sb1, xt1, p1)
                    p1 += 1
            if x1_next < 8:
                emit_x_chunk(1, xch1, x1_next, nc.sync)
                x1_next += 1
            if i == 5:
                Vhi1 = emit_sv(1, Sb1)
            elif i == 6:
                banks1 = emit_logits(1, Vhi1)

        gs1 = emit_softmax_g(1, banks1)
        o2r1 = emit_o2rows(1)
        for i in range(NP5):
            emit_conv_chunk(1, xch1, gs1, o2r1, i)

    nc.compile()
    return nc


_NC_CACHE = None


def _get_nc():
    global _NC_CACHE
    if _NC_CACHE is None:
        _NC_CACHE = build_nc()
    return _NC_CACHE


def make_in_maps(inputs):
    x = np.ascontiguousarray(np.asarray(inputs["x"], dtype=np.float32))
    wq = np.asarray(inputs["wq"], dtype=np.float32)
    wk = np.asarray(inputs["wk"], dtype=np.float32)
    wo = np.asarray(inputs["wo"], dtype=np.float32)
    bq = np.asarray(inputs["bq"], dtype=np.float32)
    bk = np.asarray(inputs["bk"], dtype=np.float32)
    bo = np.asarray(inputs["bo"], dtype=np.float32)

    x32 = x.reshape(B, C, P)
    xr = x32.astype(BF)                                   # [B, C, P] bf16
    xtr = np.ascontiguousarray(xr.transpose(0, 2, 1))     # [B, P, C] bf16
    # pair-packed xT: [B, NPG, 128, 1024], group g = p-tiles 2g, 2g+1
    xt4 = np.ascontiguousarray(
        xtr.reshape(B, NPG, 2, 128, C).transpose(0, 1, 3, 2, 4)
           .reshape(B, NPG, 128, 1024))

    # rank-2 bias-correction rows (exact f32 host math)
    s = x32.sum(axis=2)                                   # [B, C]
    qs = s @ wq.T                                         # [B, C]
    ks = s @ wk.T                                         # [B, C]

    wpack = np.zeros((128, 3 * WCOLS + 192), dtype=BF)
    for i, w in enumerate((wq, wk, wo)):
        wt = w.T.astype(BF)  # [ci, co]
        for ci in range(NCT):
            wpack[:, i * WCOLS + ci * C: i * WCOLS + (ci + 1) * C] = \
                wt[ci * 128:(ci + 1) * 128, :]
    ko = 3 * WCOLS
    wpack[:, ko + 64: ko + 192] = np.eye(128, dtype=np.float32).astype(BF)

    bpack = bo.reshape(NCT, 128).T.astype(np.float32)
    bpack = np.ascontiguousarray(bpack)

    in_maps = []
    for cix in range(NCORES):
        bias2 = np.zeros((2, BL * 2 * C), dtype=BF)
        for e in range(BL):
            ge = cix * BL + e
            bias2[0, e * 2 * C: e * 2 * C + C] = bq.astype(BF)
            bias2[1, e * 2 * C: e * 2 * C + C] = qs[ge].astype(BF)
            bias2[0, e * 2 * C + C: (e + 1) * 2 * C] = (ks[ge] + P * bk).astype(BF)
            bias2[1, e * 2 * C + C: (e + 1) * 2 * C] = bk.astype(BF)
        in_maps.append({
            "x": np.ascontiguousarray(xr[cix * BL: (cix + 1) * BL]),
            "xt": np.ascontiguousarray(xt4[cix * BL: (cix + 1) * BL]),
            "wpack": wpack, "bias2": bias2, "bpack": bpack,
        })
    return in_maps


def run_sharded(inputs, trace=False, **kw):
    nc = _get_nc()
    in_maps = make_in_maps(inputs)
    res = bass_utils.run_bass_kernel_spmd(
        nc, in_maps, core_ids=list(range(NCORES)), trace=trace, **kw
    )
    outs = [np.asarray(res.results[i]["out"]).astype(np.float32)
            for i in range(NCORES)]
    full = np.concatenate(outs, axis=0).reshape(B, C, HH, WW)
    return full.astype(np.float32), res


def kernel(**inputs):
    out, _ = run_sharded(inputs, trace=False)
    return out
